# revision 1
# baseline (speedup 1.0000x reference)
"""CAMoE GNN layer (GCNConv experts x3, softmax gating) on 8 Trainium2 cores.

Strategy (per sharding hint): nodes are sharded across the 8 cores by TARGET;
edges are partitioned by target node so the segment-sum stays core-local.
Host does only integer/index preprocessing (edge sorting, load balancing,
padding, index wrapping); all FLOPs on x / gate_features / weights run on
device.

Math: for each expert i, agg_i = A_hat @ (x @ W_i) + b_i where
A_hat = D^-1/2 (A+I) D^-1/2.  Since the aggregation is linear we aggregate
once on the input features: z = A_hat @ x, then agg_i = z @ W_i + b_i.
This cuts the per-edge gather/scatter traffic by 3x (one pass instead of one
per expert).

Device pipeline per core:
  1. dma_gather (GpSimd SWDGE) batch-gathers x[src] rows (256B) from DRAM for
     the core's edge stream (sorted by (src_chunk, target_bin)).
  2. Per 128-edge chunk: DVE builds a one-hot selection matrix
     S[e,t] = (slot(tgt[e]) == t) by comparing the shipped per-edge slot id
     against a constant iota matrix; ACT scales the gathered rows by
     dinv[src]; PE computes z_psum[t,:] += S^T @ msg (segment sum as matmul,
     accumulated in PSUM across the chunks of one target bin).
  3. Per target bin: z_acc(SBUF) += z_psum; after all 4 source-chunk passes,
     z is scaled by dinv[tgt], transposed on PE, pushed through the 3 expert
     matmuls (one [64,192] rhs), biased, ReLU'd, combined with the softmax
     gating (PE matmul K=4 + ACT exp + DVE reduce/reciprocal).
"""

import numpy as np

N = 100000
E = 1600000
D = 64
NEXP = 3
GC = 4
TEMP = 101.0
NCORES = 8
P = 128
TILES = 98                  # target bins per core
SLOTS = TILES * P           # 12544 output rows per core (>= 12500)
NBINS = NCORES * TILES      # 784 global target bins
CHUNK_SRC = 32768           # int16 gather-index limit -> 4 source chunks
NPASS = (N + CHUNK_SRC - 1) // CHUNK_SRC
SC = 64                     # max 128-edge chunks per dma_gather call (8192 idxs)

F32 = np.float32


def _host_prep(edge_index):
    """Pure index preprocessing. Returns everything needed to build per-core
    input tensors and the (input-specialized) device program."""
    src = np.concatenate([edge_index[0].astype(np.int64), np.arange(N, dtype=np.int64)])
    tgt = np.concatenate([edge_index[1].astype(np.int64), np.arange(N, dtype=np.int64)])
    deg = np.bincount(tgt, minlength=N)          # in-degree incl. self loop, >= 1
    dinv = (1.0 / np.sqrt(deg.astype(np.float64))).astype(F32)

    # --- balance target nodes into NBINS bins of <=128 slots (LPT by degree) ---
    import heapq
    order = np.argsort(-deg, kind="stable")
    heap = [(0, b) for b in range(NBINS)]
    heapq.heapify(heap)
    counts = np.zeros(NBINS, np.int64)
    node_bin = np.empty(N, np.int32)
    node_slot = np.empty(N, np.int32)
    deg_l = deg.tolist()
    for n in order.tolist():
        while True:
            load, b = heapq.heappop(heap)
            if counts[b] < P:
                break
        node_bin[n] = b
        node_slot[n] = counts[b]
        counts[b] += 1
        heapq.heappush(heap, (load + deg_l[n], b))

    # --- group edges by (target bin, source chunk) ---
    ebin = node_bin[tgt].astype(np.int64)
    epass = src // CHUNK_SRC
    key = ebin * NPASS + epass
    eorder = np.argsort(key, kind="stable")
    gcnt = np.bincount(key, minlength=NBINS * NPASS).reshape(NBINS, NPASS)
    gstart = np.zeros(NBINS * NPASS + 1, np.int64)
    np.cumsum(gcnt.reshape(-1), out=gstart[1:])
    C = -(-gcnt // P)                            # chunks per (bin, pass)

    # --- deal bins to cores so chunk-count sequences match across cores ---
    sort_idx = np.lexsort((C[:, 3], C[:, 2], C[:, 1], C[:, 0]))[::-1]
    # position j gets bins sort_idx[j*8:(j+1)*8] -> cores 0..7
    bin_of = sort_idx.reshape(TILES, NCORES)     # [pos, core] -> bin
    C_used = C[sort_idx].reshape(TILES, NCORES, NPASS).max(axis=1)  # [pos, pass]
    M_p = C_used.sum(axis=0)                     # chunks per pass (same all cores)
    M_tot = int(M_p.sum())

    # --- per-core padded edge streams ---
    srcs = src[eorder]
    slots = node_slot[tgt[eorder]].astype(np.int64)
    cores = []
    for k in range(NCORES):
        lsrc = {}
        senc = np.full(M_tot * P, -1.0, F32)     # slot id per edge (-1 = pad)
        dv = np.zeros(M_tot * P, F32)            # dinv[src] per edge
        gofs = 0                                 # global chunk offset
        gidx = []
        for p in range(NPASS):
            Lp = int(M_p[p]) * P
            ls = np.zeros(Lp, np.int64)
            cofs = 0                             # chunk offset within pass
            for j in range(TILES):
                b = bin_of[j, k]
                g = b * NPASS + p
                cnt = int(gcnt[b, p])
                if cnt:
                    sl = eorder[gstart[g]:gstart[g] + cnt]
                    dst = cofs * P + np.arange(cnt)
                    ls[dst] = src[sl] - p * CHUNK_SRC
                    senc[(gofs + cofs) * P + np.arange(cnt)] = node_slot[tgt[sl]]
                    dv[(gofs + cofs) * P + np.arange(cnt)] = dinv[src[sl]]
                cofs += int(C_used[j, p])
            assert cofs == int(M_p[p])
            gofs += cofs
            # wrap: index i lives at [i%16, i//16], replicated to 128 partitions
            w = ls.astype(np.int16).reshape(Lp // 16, 16).T
            gidx.append(np.ascontiguousarray(np.tile(w, (8, 1))))
        tgt_enc = np.ascontiguousarray(senc.reshape(M_tot, P).T.astype(np.float16))
        dsrc = np.ascontiguousarray(dv.reshape(M_tot, P).T)
        cores.append(dict(gidx=gidx, tgt_enc=tgt_enc, dsrc=dsrc))

    return dict(
        dinv=dinv, node_bin=node_bin, node_slot=node_slot, bin_of=bin_of,
        C_used=C_used, M_p=M_p, M_tot=M_tot, cores=cores,
    )


def _core_tensors(prep, k, x_parts, gate_features, W, b, Wg, consts):
    """Build the in_map (name -> np.ndarray) for core k."""
    c = prep["cores"][k]
    node_bin, node_slot = prep["node_bin"], prep["node_slot"]
    bin_of, dinv = prep["bin_of"], prep["dinv"]

    dinvt = np.zeros((P, TILES), F32)
    gft = np.zeros((GC, SLOTS), F32)
    for j in range(TILES):
        bnodes = np.nonzero(node_bin == bin_of[j, k])[0]
        sl = node_slot[bnodes]
        dinvt[sl, j] = dinv[bnodes]
        gft[:, j * P + sl] = np.asarray(gate_features)[bnodes].T

    m = dict(
        tgt_enc=c["tgt_enc"], dsrc=c["dsrc"], dinvt=dinvt, gft=gft,
        wcat=np.ascontiguousarray(np.concatenate([W[i] for i in range(NEXP)], axis=1), dtype=F32),
        wg=np.ascontiguousarray(Wg, dtype=F32),
        bbc=np.ascontiguousarray(
            np.broadcast_to(np.concatenate([b[i] for i in range(NEXP)])[None, :], (P, NEXP * D))),
        **consts,
    )
    for p in range(NPASS):
        m[f"gidx{p}"] = c["gidx"][p]
    for p, xp in enumerate(x_parts):
        m[f"x{p}"] = xp
    return m


def _build_program(C_used, M_p):
    """Build the Bass/Tile program. Only uses chunk counts (identical across
    cores), never float data."""
    import concourse.bass as bass
    import concourse.tile as tile
    from concourse import bacc, mybir

    dt = mybir.dt
    nc = bacc.Bacc("TRN2", target_bir_lowering=False, debug=False,
                   enable_asserts=False, num_devices=NCORES)

    xr = [CHUNK_SRC] * (NPASS - 1) + [N - CHUNK_SRC * (NPASS - 1)]
    x_d = [nc.dram_tensor(f"x{p}", [xr[p], D], dt.float32, kind="ExternalInput").ap()
           for p in range(NPASS)]
    gidx_d = [nc.dram_tensor(f"gidx{p}", [P, int(M_p[p]) * 8], dt.int16,
                             kind="ExternalInput").ap() for p in range(NPASS)]
    M_tot = int(M_p.sum())
    tgt_d = nc.dram_tensor("tgt_enc", [P, M_tot], dt.float16, kind="ExternalInput").ap()
    dsrc_d = nc.dram_tensor("dsrc", [P, M_tot], dt.float32, kind="ExternalInput").ap()
    dinvt_d = nc.dram_tensor("dinvt", [P, TILES], dt.float32, kind="ExternalInput").ap()
    gft_d = nc.dram_tensor("gft", [GC, SLOTS], dt.float32, kind="ExternalInput").ap()
    wcat_d = nc.dram_tensor("wcat", [D, NEXP * D], dt.float32, kind="ExternalInput").ap()
    wg_d = nc.dram_tensor("wg", [GC, NEXP], dt.float32, kind="ExternalInput").ap()
    bbc_d = nc.dram_tensor("bbc", [P, NEXP * D], dt.float32, kind="ExternalInput").ap()
    iota_d = nc.dram_tensor("iota", [P, P], dt.float16, kind="ExternalInput").ap()
    ident_d = nc.dram_tensor("ident", [P, P], dt.float32, kind="ExternalInput").ap()
    out_d = nc.dram_tensor("out", [SLOTS, D], dt.float32, kind="ExternalOutput").ap()

    # pack gather calls: whole (pass, tile) groups, <= SC chunks per call
    calls = []  # (pass, chunk_ofs_in_pass, n_chunks, [(j, C_j), ...])
    for p in range(NPASS):
        cofs = 0
        cur = []
        cur_n = 0
        for j in range(TILES):
            cj = int(C_used[j, p])
            if cj == 0:
                continue
            if cur_n + cj > SC:
                calls.append((p, cofs, cur_n, cur))
                cofs += cur_n
                cur, cur_n = [], 0
            cur.append((j, cj))
            cur_n += cj
        if cur:
            calls.append((p, cofs, cur_n, cur))

    pass_ofs = np.concatenate([[0], np.cumsum(M_p)]).astype(np.int64)

    with tile.TileContext(nc) as tc:
        with tc.tile_pool(name="const", bufs=1) as cpool, \
             tc.tile_pool(name="zacc", bufs=1) as zpool, \
             tc.tile_pool(name="msg", bufs=2) as mpool, \
             tc.tile_pool(name="meta", bufs=2) as tpool, \
             tc.tile_pool(name="work", bufs=6) as wpool, \
             tc.tile_pool(name="comb", bufs=3) as kpool, \
             tc.tile_pool(name="pz", bufs=2, space="PSUM") as pz, \
             tc.tile_pool(name="pt", bufs=1, space="PSUM") as pt, \
             tc.tile_pool(name="ph", bufs=2, space="PSUM") as ph, \
             tc.tile_pool(name="py", bufs=1, space="PSUM") as py:

            def load_const(ap_d, shape, tag, dtype=dt.float32):
                t = cpool.tile(shape, dtype, tag=tag)
                nc.sync.dma_start(t[:], ap_d)
                return t

            iota_sb = load_const(iota_d, [P, P], tag="iota", dtype=dt.float16)
            ident_sb = load_const(ident_d, [P, P], tag="ident")
            wcat_sb = load_const(wcat_d, [D, NEXP * D], tag="wcat")
            wg_sb = load_const(wg_d, [GC, NEXP], tag="wg")
            bbc_sb = load_const(bbc_d, [P, NEXP * D], tag="bbc")
            dinvt_sb = load_const(dinvt_d, [P, TILES], tag="dinvt")
            gft_sb = load_const(gft_d, [GC, SLOTS], tag="gft")

            z_acc = zpool.tile([P, TILES * D], dt.float32)
            nc.vector.memset(z_acc[:], 0.0)

            for (p, cofs, ck, groups) in calls:
                gc0 = int(pass_ofs[p]) + cofs       # global chunk offset
                msg = mpool.tile([P, SC * D], dt.float32, tag="msg")
                idx_t = tpool.tile([P, SC * 8], dt.int16, tag="idx")
                nc.sync.dma_start(idx_t[:, :ck * 8],
                                  gidx_d[p][:, cofs * 8:(cofs + ck) * 8])
                tgt_t = tpool.tile([P, SC], dt.float16, tag="tgt")
                nc.sync.dma_start(tgt_t[:, :ck], tgt_d[:, gc0:gc0 + ck])
                dv_t = tpool.tile([P, SC], dt.float32, tag="dv")
                nc.sync.dma_start(dv_t[:, :ck], dsrc_d[:, gc0:gc0 + ck])
                nc.gpsimd.dma_gather(
                    out_ap=msg[:, :ck * D].rearrange("p (c f) -> p c f", f=D),
                    in_ap=x_d[p],
                    idxs_ap=idx_t[:, :ck * 8],
                    num_idxs=ck * P,
                    num_idxs_reg=ck * P,
                    elem_size=D,
                    single_packet=False,
                )
                ci = 0                               # chunk within call
                for (j, cj) in groups:
                    z_ps = pz.tile([P, D], dt.float32, tag="zps")
                    S_g = wpool.tile([P, cj * P], dt.float16, tag="S")
                    nc.vector.tensor_tensor(
                        out=S_g[:].rearrange("p (c t) -> p c t", t=P),
                        in0=tgt_t[:, ci:ci + cj].unsqueeze(2).to_broadcast([P, cj, P]),
                        in1=iota_sb[:].unsqueeze(1).to_broadcast([P, cj, P]),
                        op=mybir.AluOpType.is_equal,
                    )
                    ms_g = wpool.tile([P, cj * D], dt.float16, tag="ms")
                    nc.vector.tensor_tensor(
                        out=ms_g[:].rearrange("p (c f) -> p c f", f=D),
                        in0=msg[:, ci * D:(ci + cj) * D].rearrange("p (c f) -> p c f", f=D),
                        in1=dv_t[:, ci:ci + cj].unsqueeze(2).to_broadcast([P, cj, D]),
                        op=mybir.AluOpType.mult,
                    )
                    for q in range(cj):
                        nc.tensor.matmul(out=z_ps[:],
                                         lhsT=S_g[:, q * P:(q + 1) * P],
                                         rhs=ms_g[:, q * D:(q + 1) * D],
                                         start=(q == 0), stop=(q == cj - 1))
                    nc.vector.tensor_add(z_acc[:, j * D:(j + 1) * D],
                                         z_acc[:, j * D:(j + 1) * D], z_ps[:])
                    ci += cj

            # ---- per-bin expert + gating phase ----
            for j in range(TILES):
                zs = kpool.tile([P, D], dt.float32, tag="zs")
                nc.scalar.mul(zs[:], z_acc[:, j * D:(j + 1) * D],
                              mul=dinvt_sb[:, j:j + 1])
                zT_ps = pt.tile([D, P], dt.float32, tag="zT")
                nc.tensor.transpose(out=zT_ps[:], in_=zs[:], identity=ident_sb[:])
                zT = kpool.tile([D, P], dt.float32, tag="zTs")
                nc.vector.tensor_copy(zT[:], zT_ps[:])
                h_ps = ph.tile([P, NEXP * D], dt.float32, tag="h")
                nc.tensor.matmul(out=h_ps[:], lhsT=zT[:], rhs=wcat_sb[:],
                                 start=True, stop=True)
                h = kpool.tile([P, NEXP * D], dt.float32, tag="hs")
                nc.vector.tensor_add(h[:], h_ps[:], bbc_sb[:])
                nc.scalar.activation(h[:], h[:], mybir.ActivationFunctionType.Relu)
                # gating
                y_ps = py.tile([P, NEXP], dt.float32, tag="y")
                nc.tensor.matmul(
                    out=y_ps[:],
                    lhsT=gft_sb[:, j * P:(j + 1) * P],
                    rhs=wg_sb[:], start=True, stop=True)
                ge = kpool.tile([P, NEXP], dt.float32, tag="ge")
                nc.scalar.activation(ge[:], y_ps[:], mybir.ActivationFunctionType.Exp,
                                     scale=float(1.0 / TEMP))
                gs = kpool.tile([P, 1], dt.float32, tag="gs")
                nc.vector.tensor_reduce(out=gs[:], in_=ge[:],
                                        axis=mybir.AxisListType.X,
                                        op=mybir.AluOpType.add)
                gr = kpool.tile([P, 1], dt.float32, tag="gr")
                nc.vector.reciprocal(gr[:], gs[:])
                acc = kpool.tile([P, D], dt.float32, tag="acc")
                tmp = kpool.tile([P, D], dt.float32, tag="tmp")
                nc.scalar.mul(acc[:], h[:, 0:D], mul=ge[:, 0:1])
                for i in range(1, NEXP):
                    nc.scalar.mul(tmp[:], h[:, i * D:(i + 1) * D], mul=ge[:, i:i + 1])
                    nc.vector.tensor_add(acc[:], acc[:], tmp[:])
                nc.vector.tensor_mul(z_acc[:, j * D:(j + 1) * D], acc[:],
                                     gr[:].to_broadcast([P, D]))
            nc.sync.dma_start(
                out_d.rearrange("(t p) f -> p t f", p=P),
                z_acc[:].rearrange("p (t f) -> p t f", f=D),
            )
    nc.finalize()
    return nc


def _consts():
    return dict(
        iota=np.ascontiguousarray(
            np.broadcast_to(np.arange(P, dtype=np.float16)[None, :], (P, P))),
        ident=np.eye(P, dtype=F32),
    )


def kernel(x, edge_index, gate_features, W, b, Wg):
    from concourse.bass_utils import run_bass_kernel_spmd

    x = np.ascontiguousarray(np.asarray(x), dtype=F32)
    edge_index = np.asarray(edge_index)
    prep = _host_prep(edge_index)
    x_parts = [np.ascontiguousarray(x[p * CHUNK_SRC:min((p + 1) * CHUNK_SRC, N)])
               for p in range(NPASS)]
    consts = _consts()
    in_maps = [_core_tensors(prep, k, x_parts, gate_features, W, b, Wg, consts)
               for k in range(NCORES)]
    nc = _build_program(prep["C_used"], prep["M_p"])
    res = run_bass_kernel_spmd(nc, in_maps, core_ids=list(range(NCORES)))
    global LAST_RESULTS
    LAST_RESULTS = res
    node_bin, node_slot = prep["node_bin"], prep["node_slot"]
    bin_of = prep["bin_of"]
    # bin -> (core, pos)
    bin_core = np.empty(NBINS, np.int64)
    bin_pos = np.empty(NBINS, np.int64)
    for j in range(TILES):
        for k in range(NCORES):
            bin_core[bin_of[j, k]] = k
            bin_pos[bin_of[j, k]] = j
    out = np.empty((N, D), F32)
    rows = bin_pos[node_bin] * P + node_slot
    per_core = np.stack([res.results[k]["out"] for k in range(NCORES)])
    out[:] = per_core[bin_core[node_bin], rows]
    return out



# revision 2
# speedup vs baseline: 1.1566x; 1.1566x over previous
"""CAMoE GNN layer (GCNConv experts x3, softmax gating) on 8 Trainium2 cores.

Strategy (per sharding hint): nodes are sharded across the 8 cores by TARGET;
edges are partitioned by target node so the segment-sum stays core-local.
Host does only integer/index preprocessing (edge sorting, load balancing,
padding, index wrapping); all FLOPs on x / gate_features / weights run on
device.

Math: for each expert i, agg_i = A_hat @ (x @ W_i) + b_i where
A_hat = D^-1/2 (A+I) D^-1/2.  Since the aggregation is linear we aggregate
once on the input features: z = A_hat @ x, then agg_i = z @ W_i + b_i.
This cuts the per-edge gather/scatter traffic by 3x (one pass instead of one
per expert).

Device pipeline per core:
  1. dma_gather (GpSimd SWDGE) batch-gathers x[src] rows (256B) from DRAM for
     the core's edge stream (sorted by (src_chunk, target_bin)).
  2. Per 128-edge chunk: DVE builds a one-hot selection matrix
     S[e,t] = (slot(tgt[e]) == t) by comparing the shipped per-edge slot id
     against a constant iota matrix; ACT scales the gathered rows by
     dinv[src]; PE computes z_psum[t,:] += S^T @ msg (segment sum as matmul,
     accumulated in PSUM across the chunks of one target bin).
  3. Per target bin: z_acc(SBUF) += z_psum; after all 4 source-chunk passes,
     z is scaled by dinv[tgt], transposed on PE, pushed through the 3 expert
     matmuls (one [64,192] rhs), biased, ReLU'd, combined with the softmax
     gating (PE matmul K=4 + ACT exp + DVE reduce/reciprocal).
"""

import numpy as np

N = 100000
E = 1600000
D = 64
NEXP = 3
GC = 4
TEMP = 101.0
NCORES = 8
P = 128
TILES = 102                 # target bins per core
CAP = 123                   # max nodes per bin (slots used)
SLOTS = TILES * P           # output rows per core
NBINS = NCORES * TILES      # 784 global target bins
CHUNK_SRC = 32768           # int16 gather-index limit -> 4 source chunks
NPASS = (N + CHUNK_SRC - 1) // CHUNK_SRC
SC = 64                     # max 128-edge chunks per dma_gather call (8192 idxs)

F32 = np.float32


def _host_prep(edge_index):
    """Pure index preprocessing. Returns everything needed to build per-core
    input tensors and the (input-specialized) device program."""
    src = edge_index[0].astype(np.int64)
    tgt = edge_index[1].astype(np.int64)
    deg = np.bincount(tgt, minlength=N) + 1      # in-degree incl. self loop, >= 1
    dinv = (1.0 / np.sqrt(deg.astype(np.float64))).astype(F32)
    indeg = deg - 1                              # gathered edges per target

    # --- balance target nodes into NBINS bins of <=128 slots (LPT by degree) ---
    import heapq
    order = np.argsort(-indeg, kind="stable")
    heap = [(0, b) for b in range(NBINS)]
    heapq.heapify(heap)
    counts = np.zeros(NBINS, np.int64)
    node_bin = np.empty(N, np.int32)
    node_slot = np.empty(N, np.int32)
    deg_l = indeg.tolist()
    for n in order.tolist():
        while True:
            load, b = heapq.heappop(heap)
            if counts[b] < CAP:
                break
        node_bin[n] = b
        node_slot[n] = counts[b]
        counts[b] += 1
        heapq.heappush(heap, (load + deg_l[n], b))

    # --- group edges by (target bin, source chunk) ---
    ebin = node_bin[tgt].astype(np.int64)
    epass = src // CHUNK_SRC
    key = ebin * NPASS + epass
    eorder = np.argsort(key, kind="stable")
    gcnt = np.bincount(key, minlength=NBINS * NPASS).reshape(NBINS, NPASS)
    gstart = np.zeros(NBINS * NPASS + 1, np.int64)
    np.cumsum(gcnt.reshape(-1), out=gstart[1:])
    C = -(-gcnt // P)                            # chunks per (bin, pass)

    # --- deal bins to cores so chunk-count sequences match across cores ---
    sort_idx = np.lexsort((C[:, 3], C[:, 2], C[:, 1], C[:, 0]))[::-1]
    # position j gets bins sort_idx[j*8:(j+1)*8] -> cores 0..7
    bin_of = sort_idx.reshape(TILES, NCORES)     # [pos, core] -> bin
    C_used = C[sort_idx].reshape(TILES, NCORES, NPASS).max(axis=1)  # [pos, pass]
    M_p = C_used.sum(axis=0)                     # chunks per pass (same all cores)
    M_tot = int(M_p.sum())

    # --- per-core padded edge streams ---
    srcs = src[eorder]
    slots = node_slot[tgt[eorder]].astype(np.int64)
    cores = []
    for k in range(NCORES):
        lsrc = {}
        senc = np.full(M_tot * P, -1.0, F32)     # slot id per edge (-1 = pad)
        dv = np.zeros(M_tot * P, F32)            # dinv[src] per edge
        gofs = 0                                 # global chunk offset
        gidx = []
        for p in range(NPASS):
            Lp = int(M_p[p]) * P
            ls = np.zeros(Lp, np.int64)
            cofs = 0                             # chunk offset within pass
            for j in range(TILES):
                b = bin_of[j, k]
                g = b * NPASS + p
                cnt = int(gcnt[b, p])
                if cnt:
                    sl = eorder[gstart[g]:gstart[g] + cnt]
                    dst = cofs * P + np.arange(cnt)
                    ls[dst] = src[sl] - p * CHUNK_SRC
                    senc[(gofs + cofs) * P + np.arange(cnt)] = node_slot[tgt[sl]]
                    dv[(gofs + cofs) * P + np.arange(cnt)] = dinv[src[sl]]
                cofs += int(C_used[j, p])
            assert cofs == int(M_p[p])
            gofs += cofs
            # wrap: index i lives at [i%16, i//16], replicated to 128 partitions
            w = ls.astype(np.int16).reshape(Lp // 16, 16).T
            gidx.append(np.ascontiguousarray(np.tile(w, (8, 1))))
        tgt_enc = np.ascontiguousarray(senc.reshape(M_tot, P).T.astype(np.float16))
        dsrc = np.ascontiguousarray(dv.reshape(M_tot, P).T)
        cores.append(dict(gidx=gidx, tgt_enc=tgt_enc, dsrc=dsrc))

    return dict(
        dinv=dinv, node_bin=node_bin, node_slot=node_slot, bin_of=bin_of,
        C_used=C_used, M_p=M_p, M_tot=M_tot, cores=cores,
    )


def _core_tensors(prep, k, x_np, x_parts, gate_features, W, b, Wg, consts):
    """Build the in_map (name -> np.ndarray) for core k."""
    c = prep["cores"][k]
    node_bin, node_slot = prep["node_bin"], prep["node_slot"]
    bin_of, dinv = prep["bin_of"], prep["dinv"]

    dinvt = np.zeros((P, TILES), F32)
    gft = np.zeros((GC, SLOTS), F32)
    xself = np.zeros((P, TILES * 64), F32)
    for j in range(TILES):
        bnodes = np.nonzero(node_bin == bin_of[j, k])[0]
        sl = node_slot[bnodes]
        dinvt[sl, j] = dinv[bnodes]
        gft[:, j * P + sl] = np.asarray(gate_features)[bnodes].T
        xself[sl, j * 64:(j + 1) * 64] = x_np[bnodes]

    m = dict(
        tgt_enc=c["tgt_enc"], dsrc=c["dsrc"], dinvt=dinvt, gft=gft, xself=xself,
        wcat=np.ascontiguousarray(np.concatenate([W[i] for i in range(NEXP)], axis=1), dtype=F32),
        wg=np.ascontiguousarray(Wg, dtype=F32),
        bbc=np.ascontiguousarray(
            np.broadcast_to(np.concatenate([b[i] for i in range(NEXP)])[None, :], (P, NEXP * D))),
        **consts,
    )
    for p in range(NPASS):
        m[f"gidx{p}"] = c["gidx"][p]
    for p, xp in enumerate(x_parts):
        m[f"x{p}"] = xp
    return m


def _build_program(C_used, M_p):
    """Build the Bass/Tile program. Only uses chunk counts (identical across
    cores), never float data."""
    import concourse.bass as bass
    import concourse.tile as tile
    from concourse import bacc, mybir

    dt = mybir.dt
    nc = bacc.Bacc("TRN2", target_bir_lowering=False, debug=False,
                   enable_asserts=False, num_devices=NCORES)

    xr = [CHUNK_SRC] * (NPASS - 1) + [N - CHUNK_SRC * (NPASS - 1)]
    x_d = [nc.dram_tensor(f"x{p}", [xr[p], D], dt.float32, kind="ExternalInput").ap()
           for p in range(NPASS)]
    gidx_d = [nc.dram_tensor(f"gidx{p}", [P, int(M_p[p]) * 8], dt.int16,
                             kind="ExternalInput").ap() for p in range(NPASS)]
    M_tot = int(M_p.sum())
    tgt_d = nc.dram_tensor("tgt_enc", [P, M_tot], dt.float16, kind="ExternalInput").ap()
    dsrc_d = nc.dram_tensor("dsrc", [P, M_tot], dt.float32, kind="ExternalInput").ap()
    dinvt_d = nc.dram_tensor("dinvt", [P, TILES], dt.float32, kind="ExternalInput").ap()
    xself_d = nc.dram_tensor("xself", [P, TILES * 64], dt.float32, kind="ExternalInput").ap()
    gft_d = nc.dram_tensor("gft", [GC, SLOTS], dt.float32, kind="ExternalInput").ap()
    wcat_d = nc.dram_tensor("wcat", [D, NEXP * D], dt.float32, kind="ExternalInput").ap()
    wg_d = nc.dram_tensor("wg", [GC, NEXP], dt.float32, kind="ExternalInput").ap()
    bbc_d = nc.dram_tensor("bbc", [P, NEXP * D], dt.float32, kind="ExternalInput").ap()
    iota_d = nc.dram_tensor("iota", [P, P], dt.float16, kind="ExternalInput").ap()
    ident_d = nc.dram_tensor("ident", [P, P], dt.float32, kind="ExternalInput").ap()
    out_d = nc.dram_tensor("out", [SLOTS, D], dt.float32, kind="ExternalOutput").ap()

    # pack gather calls: whole (pass, tile) groups, <= SC chunks per call
    # process the largest pass LAST so per-bin phase-2 work interleaves
    # under the tail of descriptor generation
    pass_order = sorted(range(NPASS), key=lambda p: int(M_p[p]))
    calls = []  # (pass, chunk_ofs_in_pass, n_chunks, [(j, C_j), ...])
    for p in pass_order:
        cofs = 0
        cur = []
        cur_n = 0
        for j in range(TILES):
            cj = int(C_used[j, p])
            if cj == 0:
                continue
            if cur_n + cj > SC:
                calls.append((p, cofs, cur_n, cur))
                cofs += cur_n
                cur, cur_n = [], 0
            cur.append((j, cj))
            cur_n += cj
        if cur:
            calls.append((p, cofs, cur_n, cur))

    pass_ofs = np.concatenate([[0], np.cumsum(M_p)]).astype(np.int64)

    with tile.TileContext(nc) as tc:
        with tc.tile_pool(name="const", bufs=1) as cpool, \
             tc.tile_pool(name="zacc", bufs=1) as zpool, \
             tc.tile_pool(name="msg", bufs=2) as mpool, \
             tc.tile_pool(name="meta", bufs=2) as tpool, \
             tc.tile_pool(name="work", bufs=6) as wpool, \
             tc.tile_pool(name="comb", bufs=3) as kpool, \
             tc.tile_pool(name="pz", bufs=2, space="PSUM") as pz, \
             tc.tile_pool(name="pt", bufs=1, space="PSUM") as pt, \
             tc.tile_pool(name="ph", bufs=2, space="PSUM") as ph, \
             tc.tile_pool(name="py", bufs=1, space="PSUM") as py:

            def load_const(ap_d, shape, tag, dtype=dt.float32):
                t = cpool.tile(shape, dtype, tag=tag)
                nc.sync.dma_start(t[:], ap_d)
                return t

            iota_sb = load_const(iota_d, [P, P], tag="iota", dtype=dt.float16)
            ident_sb = load_const(ident_d, [P, P], tag="ident")
            wcat_sb = load_const(wcat_d, [D, NEXP * D], tag="wcat")
            wg_sb = load_const(wg_d, [GC, NEXP], tag="wg")
            bbc_sb = load_const(bbc_d, [P, NEXP * D], tag="bbc")
            dinvt_sb = load_const(dinvt_d, [P, TILES], tag="dinvt")
            xself_sb = load_const(xself_d, [P, TILES * D], tag="xself")
            gft_sb = load_const(gft_d, [GC, SLOTS], tag="gft")

            z_acc = zpool.tile([P, TILES * D], dt.float32)
            nc.vector.tensor_tensor(
                out=z_acc[:].rearrange("p (t f) -> p t f", f=D),
                in0=xself_sb[:].rearrange("p (t f) -> p t f", f=D),
                in1=dinvt_sb[:].unsqueeze(2).to_broadcast([P, TILES, D]),
                op=mybir.AluOpType.mult,
            )

            for (p, cofs, ck, groups) in calls:
                gc0 = int(pass_ofs[p]) + cofs       # global chunk offset
                msg = mpool.tile([P, SC * D], dt.float32, tag="msg")
                idx_t = tpool.tile([P, SC * 8], dt.int16, tag="idx")
                nc.sync.dma_start(idx_t[:, :ck * 8],
                                  gidx_d[p][:, cofs * 8:(cofs + ck) * 8])
                tgt_t = tpool.tile([P, SC], dt.float16, tag="tgt")
                nc.sync.dma_start(tgt_t[:, :ck], tgt_d[:, gc0:gc0 + ck])
                dv_t = tpool.tile([P, SC], dt.float32, tag="dv")
                nc.sync.dma_start(dv_t[:, :ck], dsrc_d[:, gc0:gc0 + ck])
                nc.gpsimd.dma_gather(
                    out_ap=msg[:, :ck * D].rearrange("p (c f) -> p c f", f=D),
                    in_ap=x_d[p],
                    idxs_ap=idx_t[:, :ck * 8],
                    num_idxs=ck * P,
                    num_idxs_reg=ck * P,
                    elem_size=D,
                    single_packet=False,
                )
                ci = 0                               # chunk within call
                for (j, cj) in groups:
                    z_ps = pz.tile([P, D], dt.float32, tag="zps")
                    S_g = wpool.tile([P, cj * P], dt.float16, tag="S")
                    nc.vector.tensor_tensor(
                        out=S_g[:].rearrange("p (c t) -> p c t", t=P),
                        in0=tgt_t[:, ci:ci + cj].unsqueeze(2).to_broadcast([P, cj, P]),
                        in1=iota_sb[:].unsqueeze(1).to_broadcast([P, cj, P]),
                        op=mybir.AluOpType.is_equal,
                    )
                    ms_g = wpool.tile([P, cj * D], dt.float16, tag="ms")
                    nc.vector.tensor_tensor(
                        out=ms_g[:].rearrange("p (c f) -> p c f", f=D),
                        in0=msg[:, ci * D:(ci + cj) * D].rearrange("p (c f) -> p c f", f=D),
                        in1=dv_t[:, ci:ci + cj].unsqueeze(2).to_broadcast([P, cj, D]),
                        op=mybir.AluOpType.mult,
                    )
                    for q in range(cj):
                        nc.tensor.matmul(out=z_ps[:],
                                         lhsT=S_g[:, q * P:(q + 1) * P],
                                         rhs=ms_g[:, q * D:(q + 1) * D],
                                         start=(q == 0), stop=(q == cj - 1))
                    nc.vector.tensor_add(z_acc[:, j * D:(j + 1) * D],
                                         z_acc[:, j * D:(j + 1) * D], z_ps[:])
                    if p == last_pass:
                        phase2(j)
                    ci += cj

            last_pass = pass_order[-1]

            def phase2(j):
                zs = kpool.tile([P, D], dt.float32, tag="zs")
                nc.scalar.mul(zs[:], z_acc[:, j * D:(j + 1) * D],
                              mul=dinvt_sb[:, j:j + 1])
                zT_ps = pt.tile([D, P], dt.float32, tag="zT")
                nc.tensor.transpose(out=zT_ps[:], in_=zs[:], identity=ident_sb[:])
                zT = kpool.tile([D, P], dt.float32, tag="zTs")
                nc.vector.tensor_copy(zT[:], zT_ps[:])
                h_ps = ph.tile([P, NEXP * D], dt.float32, tag="h")
                nc.tensor.matmul(out=h_ps[:], lhsT=zT[:], rhs=wcat_sb[:],
                                 start=True, stop=True)
                h = kpool.tile([P, NEXP * D], dt.float32, tag="hs")
                nc.vector.tensor_add(h[:], h_ps[:], bbc_sb[:])
                nc.scalar.activation(h[:], h[:], mybir.ActivationFunctionType.Relu)
                # gating
                y_ps = py.tile([P, NEXP], dt.float32, tag="y")
                nc.tensor.matmul(
                    out=y_ps[:],
                    lhsT=gft_sb[:, j * P:(j + 1) * P],
                    rhs=wg_sb[:], start=True, stop=True)
                ge = kpool.tile([P, NEXP], dt.float32, tag="ge")
                nc.scalar.activation(ge[:], y_ps[:], mybir.ActivationFunctionType.Exp,
                                     scale=float(1.0 / TEMP))
                gs = kpool.tile([P, 1], dt.float32, tag="gs")
                nc.vector.tensor_reduce(out=gs[:], in_=ge[:],
                                        axis=mybir.AxisListType.X,
                                        op=mybir.AluOpType.add)
                gr = kpool.tile([P, 1], dt.float32, tag="gr")
                nc.vector.reciprocal(gr[:], gs[:])
                acc = kpool.tile([P, D], dt.float32, tag="acc")
                tmp = kpool.tile([P, D], dt.float32, tag="tmp")
                nc.scalar.mul(acc[:], h[:, 0:D], mul=ge[:, 0:1])
                for i in range(1, NEXP):
                    nc.scalar.mul(tmp[:], h[:, i * D:(i + 1) * D], mul=ge[:, i:i + 1])
                    nc.vector.tensor_add(acc[:], acc[:], tmp[:])
                nc.vector.tensor_mul(z_acc[:, j * D:(j + 1) * D], acc[:],
                                     gr[:].to_broadcast([P, D]))
            nc.sync.dma_start(
                out_d.rearrange("(t p) f -> p t f", p=P),
                z_acc[:].rearrange("p (t f) -> p t f", f=D),
            )
    nc.finalize()
    return nc


def _consts():
    return dict(
        iota=np.ascontiguousarray(
            np.broadcast_to(np.arange(P, dtype=np.float16)[None, :], (P, P))),
        ident=np.eye(P, dtype=F32),
    )


def kernel(x, edge_index, gate_features, W, b, Wg):
    from concourse.bass_utils import run_bass_kernel_spmd

    x = np.ascontiguousarray(np.asarray(x), dtype=F32)
    edge_index = np.asarray(edge_index)
    prep = _host_prep(edge_index)
    x_parts = [np.ascontiguousarray(x[p * CHUNK_SRC:min((p + 1) * CHUNK_SRC, N)])
               for p in range(NPASS)]
    consts = _consts()
    in_maps = [_core_tensors(prep, k, x, x_parts, gate_features, W, b, Wg, consts)
               for k in range(NCORES)]
    nc = _build_program(prep["C_used"], prep["M_p"])
    res = run_bass_kernel_spmd(nc, in_maps, core_ids=list(range(NCORES)))
    global LAST_RESULTS
    LAST_RESULTS = res
    node_bin, node_slot = prep["node_bin"], prep["node_slot"]
    bin_of = prep["bin_of"]
    # bin -> (core, pos)
    bin_core = np.empty(NBINS, np.int64)
    bin_pos = np.empty(NBINS, np.int64)
    for j in range(TILES):
        for k in range(NCORES):
            bin_core[bin_of[j, k]] = k
            bin_pos[bin_of[j, k]] = j
    out = np.empty((N, D), F32)
    rows = bin_pos[node_bin] * P + node_slot
    per_core = np.stack([res.results[k]["out"] for k in range(NCORES)])
    out[:] = per_core[bin_core[node_bin], rows]
    return out

